# revision 37
# baseline (speedup 1.0000x reference)
"""AdaFace loss on 8 TRN2 NeuronCores, class-parallel.

Baseline skeleton (one PSUM consumer per tile keeps the PE at full
clock): shard 100k weight rows across 8 cores, fp8e4 DoubleRow matmuls
with the whole weight shard resident in SBUF, fixed log-softmax shift
of 32 (|logit| <= 32) so no max pass / collective is needed.

v5 offload: per batch chunk, one of the six 2048-wide class tiles is
drained by the Vector engine as a Schraudolph fake-exp (one
tensor_scalar affine f32->int16; the int16 bit pattern IS bf16(exp)),
instead of by ScalarE. GpSimd absorbs one accumulate (eacc2 = fake +
one ScalarE tile) so the Vector engine's running-add chain stays flat.
ScalarE drops from 28 to 24 activations. Host folds the fake-exp mean
bias into the affine constant (rho de-bias) and does the O(B) combine:
margin-target correction, ln, weighted mean.
"""

import numpy as np
import ml_dtypes

import concourse.bass as bass
import concourse.tile as tile
from concourse import bacc, mybir
from concourse.bass_utils import run_bass_kernel_spmd

B = 512
D = 256
C = 100000
NCORES = 8
CSH = C // NCORES          # 12500 classes per core
CPAD = 12544
NPAD_TOT = (CPAD - CSH) * NCORES

M0 = 0.5
M_MIN = 0.25
SCALE = 32.0
SHIFT = 32.0               # fixed log-softmax shift (|logits| <= SCALE)
FP8_PRESCALE = 8.0         # both operands scaled by 8 -> matmul gives 64*cos

# tile order per chunk: the fake-exp tile sits early (its vector-engine
# drain overlaps ScalarE finishing the previous chunk); the small tile
# leads (cheap starter) except in the last chunk where it trails so the
# final reduce tail is short
FAKE_C0 = 2 * 2048          # class offset of the fake-drained tile
BIGS = [0, 2048, 6144, 8192, 10240]
TILES_NONLAST = [(12288, 256), (FAKE_C0, 2048)] + [(c, 2048) for c in BIGS]
TILES_LAST = [(FAKE_C0, 2048)] + [(c, 2048) for c in BIGS] + [(12288, 256)]

# Schraudolph constants: from x = 64*cos want bf16 bits of
# exp(0.5*x - 32): i16 = rint(x*FA + FB_EFF)
LOG2E = 1.4426950408889634
FA = 64.0 * LOG2E
FB = 16256.0 - 4096.0 * LOG2E

f32 = mybir.dt.float32
bf16 = mybir.dt.bfloat16
i16 = mybir.dt.int16
fp8 = mybir.dt.float8e4

NBC = B // 128             # 4 batch chunks

_cached_nc = None
_last_results = None


def _schraudolph_rho(fb):
    """Mean ratio fake_exp/exp over the logit range (HW rounds to
    nearest on the f32->i16 convert)."""
    t = np.linspace(-60.0, -1.0, 200001)
    x = (t + 32.0) * 2.0
    y = np.float32(x) * np.float32(FA) + np.float32(fb)
    i = np.rint(y).astype(np.int16)
    v = i.view(ml_dtypes.bfloat16).astype(np.float64)
    return float(np.mean(v / np.exp(t)))


# value ~ 2^((i-16256)/128): dividing by rho shifts the constant by
# -128*log2(rho)
FB_EFF = FB - 128.0 * np.log2(_schraudolph_rho(FB))
FB_EFF = FB_EFF - 128.0 * np.log2(_schraudolph_rho(FB_EFF))


def _build():
    global _cached_nc
    if _cached_nc is not None:
        return _cached_nc

    nc = bacc.Bacc(
        "TRN2", target_bir_lowering=False, debug=False, num_devices=NCORES
    )

    # [p, c, j] pair-interleaved fp8: contraction index k = j*128 + p
    wnT_d = nc.dram_tensor("wnT", [128, CPAD, 2], fp8, kind="ExternalInput")
    featnT_d = nc.dram_tensor("featnT", [128, 2, B], fp8, kind="ExternalInput")
    out_d = nc.dram_tensor("out", [128, NBC], f32, kind="ExternalOutput")

    with tile.TileContext(nc) as tc:
        with (
            tc.tile_pool(name="persist", bufs=1) as persist,
            tc.tile_pool(name="epool", bufs=3) as epool,
            tc.tile_pool(name="psum", bufs=2, space="PSUM") as psum,
        ):
            fsb = persist.tile([128, 2, B], fp8)
            nc.sync.dma_start(out=fsb[:], in_=featnT_d[:])

            wsb = persist.tile([128, CPAD, 2], fp8)
            # chunked loads in consumption order; sync's queue is free
            # earliest (shortest preamble), scalar joins after its
            # activation-table load
            plan = [
                (nc.sync, 12288, 12544),
                (nc.sync, 0, 1536),
                (nc.scalar, 1536, 3072),
                (nc.sync, 3072, 4608),
                (nc.scalar, 4608, 6144),
                (nc.sync, 6144, 7680),
                (nc.scalar, 7680, 9216),
                (nc.sync, 9216, 10752),
                (nc.scalar, 10752, 12288),
            ]
            for eng, lo, hi in plan:
                eng.dma_start(out=wsb[:, lo:hi, :], in_=wnT_d[:, lo:hi, :])

            bias_s = persist.tile([128, 1], f32)
            nc.gpsimd.memset(bias_s[:], -SHIFT)

            eacc = [
                persist.tile(
                    [128, 2048], bf16, tag=f"eacc{bc}", name=f"eacc{bc}"
                )
                for bc in range(NBC)
            ]

            S_all = persist.tile([128, NBC], f32)
            S_main = persist.tile([128, 1], f32)

            S_small = persist.tile([128, 1], f32)

            # roles per chunk (non-last):
            #   small -> ScalarE, stashed, joins eacc[:, :256] at b4
            #   FAKE  -> vector-engine Schraudolph drain (fi)
            #   b0,b1 -> ScalarE; gpsimd folds ea3 = b0 + b1
            #   b3    -> ScalarE; gpsimd folds ea2 = fi + b3 (lands late,
            #            folded into eacc at the START of the next chunk)
            #   b4    -> ScalarE, copy starts eacc
            #   b5    -> ScalarE; vector adds b5 + ea3
            # last chunk: no ea2 (gpsimd only folds fi + b3), vector adds
            # b1/b4 directly, split final reduce keeps the tail short
            ea2_prev = None
            for bc in range(NBC):
                last = bc == NBC - 1
                tiles = TILES_LAST if last else TILES_NONLAST
                lhs = fsb[:, :, bc * 128:(bc + 1) * 128]
                fi = epool.tile([128, 2048], i16, tag="fi")
                ea2 = epool.tile([128, 2048], bf16, tag="ea2")
                ea3 = epool.tile([128, 2048], bf16, tag="ea3")
                esc_by_c0 = {}
                for ti, (c0, cw) in enumerate(tiles):
                    ps = psum.tile([128, 2048], f32, tag="ps")
                    for j in range(0, cw, 512):
                        jw = min(512, cw - j)
                        nc.tensor.matmul(
                            ps[:, j:j + jw],
                            lhs,
                            wsb[:, c0 + j:c0 + j + jw, :].transpose([0, 2, 1]),
                            start=True, stop=True,
                            perf_mode=mybir.MatmulPerfMode.DoubleRow,
                        )
                    if c0 == FAKE_C0:
                        nc.vector.tensor_scalar(
                            fi[:], ps[:],
                            FA, FB_EFF,
                            mybir.AluOpType.mult, mybir.AluOpType.add,
                        )
                        if ea2_prev is not None:
                            # deferred fold of the previous chunk's late
                            # gpsimd output
                            nc.vector.tensor_add(
                                eacc[bc - 1][:], eacc[bc - 1][:], ea2_prev[:]
                            )
                        continue
                    esc = epool.tile([128, cw], bf16, tag=f"esc{c0}")
                    esc_by_c0[c0] = esc
                    nc.scalar.activation(
                        esc[:], ps[:, :cw],
                        mybir.ActivationFunctionType.Exp,
                        bias=bias_s[:], scale=SCALE / (FP8_PRESCALE**2),
                    )
                    if c0 == 12288:          # small tile
                        if last:
                            nc.vector.tensor_add(
                                eacc[bc][:, :cw], eacc[bc][:, :cw], esc[:]
                            )
                        # non-last: stashed, added at b4
                    elif c0 == 0:            # b0
                        if last:
                            nc.vector.tensor_copy(eacc[bc][:], esc[:])
                    elif c0 == 2048:         # b1
                        if last:
                            nc.vector.tensor_add(
                                eacc[bc][:], eacc[bc][:], esc[:]
                            )
                        else:
                            nc.gpsimd.tensor_tensor(
                                ea3[:], esc_by_c0[0][:], esc[:],
                                mybir.AluOpType.add,
                            )
                    elif c0 == 6144:         # b3
                        nc.gpsimd.tensor_tensor(
                            ea2[:], fi[:].bitcast(bf16), esc[:],
                            mybir.AluOpType.add,
                        )
                    elif c0 == 8192:         # b4
                        if last:
                            nc.vector.tensor_add(
                                eacc[bc][:], eacc[bc][:], esc[:]
                            )
                        else:
                            nc.vector.tensor_copy(eacc[bc][:], esc[:])
                            nc.vector.tensor_add(
                                eacc[bc][:, :256], eacc[bc][:, :256],
                                esc_by_c0[12288][:],
                            )
                    elif c0 == 10240:        # b5
                        nc.vector.tensor_add(eacc[bc][:], eacc[bc][:], esc[:])
                        if last:
                            nc.vector.tensor_add(
                                eacc[bc][:], eacc[bc][:], ea2[:]
                            )
                            nc.vector.tensor_reduce(
                                S_main[:],
                                eacc[bc][:, 256:],
                                axis=mybir.AxisListType.X,
                                op=mybir.AluOpType.add,
                            )
                        else:
                            nc.vector.tensor_add(
                                eacc[bc][:], eacc[bc][:], ea3[:]
                            )
                    if bc > 0 and ti == 2:
                        nc.vector.tensor_reduce(
                            S_all[:, bc - 1:bc],
                            eacc[bc - 1][:],
                            axis=mybir.AxisListType.X,
                            op=mybir.AluOpType.add,
                        )
                ea2_prev = None if last else ea2

            nc.vector.tensor_reduce(
                S_small[:],
                eacc[NBC - 1][:, :256],
                axis=mybir.AxisListType.X,
                op=mybir.AluOpType.add,
            )
            nc.vector.tensor_add(S_all[:, NBC - 1:NBC], S_main[:], S_small[:])
            nc.sync.dma_start(out=out_d[:], in_=S_all[:])

    nc.compile()
    _cached_nc = nc
    return nc


def _host_prep(features, weight, weights, labels):
    """Everything O(B*D) / O(C*D) that is not the big matmul."""
    f = features.astype(np.float64)
    norms = np.sqrt((f * f).sum(axis=1))
    lo, hi = norms.min(), norms.max()
    denom = max(hi - lo, 1e-8)
    margins = np.clip(M_MIN + (M0 - M_MIN) * (norms - lo) / denom, M_MIN, M0)
    feat_n = f / np.maximum(norms, 1e-12)[:, None]

    wlab = weight[labels].astype(np.float64)
    wlab_n = wlab / np.maximum(
        np.sqrt((wlab * wlab).sum(axis=1)), 1e-12
    )[:, None]
    cos_t = np.clip((feat_n * wlab_n).sum(axis=1), -1.0 + 1e-7, 1.0 - 1e-7)
    cos_m = cos_t * np.cos(margins) - np.sqrt(1.0 - cos_t * cos_t) * np.sin(
        margins
    )
    t_logit = SCALE * cos_m
    corr = (
        np.exp(SCALE * cos_m - SHIFT)
        - np.exp(SCALE * cos_t - SHIFT)
        - NPAD_TOT * np.exp(-SHIFT)
    )
    coef = weights.astype(np.float64) / B
    return feat_n, corr, coef, t_logit


def _to_dr_layout(mat_t, width):
    """[D, X] f32 -> [128, X, 2] fp8, pair-interleaved, k = j*128 + p."""
    a = mat_t.reshape(2, 128, width)          # [j, p, X]
    a = np.ascontiguousarray(a.transpose(1, 2, 0))  # [p, X, j]
    return a.astype(ml_dtypes.float8_e4m3)


def kernel(features, weight, weights, labels):
    global _last_results
    features = np.asarray(features, dtype=np.float32)
    weight = np.asarray(weight, dtype=np.float32)
    weights = np.asarray(weights, dtype=np.float32)
    labels = np.asarray(labels).astype(np.int64)

    feat_n, corr, coef, t_logit = _host_prep(features, weight, weights, labels)

    wn = weight / np.maximum(
        np.linalg.norm(weight, axis=1, keepdims=True), 1e-12
    )
    featnT = np.ascontiguousarray(feat_n.T.astype(np.float32)) * FP8_PRESCALE
    a = featnT.reshape(2, 128, B)
    featnT8 = np.ascontiguousarray(a.transpose(1, 0, 2)).astype(
        ml_dtypes.float8_e4m3
    )

    in_maps = []
    for i in range(NCORES):
        sh = wn[i * CSH:(i + 1) * CSH]  # [CSH, D]
        wt = np.zeros((D, CPAD), dtype=np.float32)
        wt[:, :CSH] = sh.T * FP8_PRESCALE
        in_maps.append(
            {"wnT": _to_dr_layout(wt, CPAD), "featnT": featnT8}
        )

    nc = _build()
    res = run_bass_kernel_spmd(nc, in_maps, list(range(NCORES)))
    _last_results = res

    # ---- host combine ----
    S = np.zeros(B, dtype=np.float64)
    for i in range(NCORES):
        sc = np.asarray(res.results[i]["out"], dtype=np.float64)  # [128, 4]
        for bc in range(NBC):
            S[bc * 128:(bc + 1) * 128] += sc[:, bc]

    Z = S + corr
    per = SHIFT + np.log(Z) - t_logit
    loss = float((coef * per).sum())
    return np.array(loss, dtype=np.float32)


# revision 38
# speedup vs baseline: 1.1420x; 1.1420x over previous
"""AdaFace loss on 8 TRN2 NeuronCores, class-parallel.

Strategy: shard the 100k weight rows (classes) across 8 cores. Host
pre-normalizes rows, transposes to [D, C_shard], scales by 8 and casts to
fp8e4 (scale keeps values out of the e4m3 subnormal range; the ScalarE
exp absorbs it: exp(0.5*x - 32) of the 64*cos matmul result). Since
|logit| <= 32, a fixed shift of 32 replaces the per-row max of a
standard log-softmax, so no max collective is needed. Each core returns
per-batch-chunk partial sums of exp(32c-32); the host does the final
O(B) combine: sum across cores, margin-target correction (cos(theta+m)
needs only sqrt, no arccos), ln, weighted dot. No device collective.

Device per core: the whole fp8 weight shard stays resident in SBUF
(24.5KB/partition); DoubleRow matmuls (K=256 per instruction) fill
4-bank PSUM tiles [128b, 2048c]; ScalarE exp -> bf16; VectorE 2x-mode
running adds + one final reduce per batch chunk; single DMA out.
"""

import numpy as np
import ml_dtypes

import concourse.bass as bass
import concourse.tile as tile
from concourse import bacc, mybir
from concourse.bass_utils import run_bass_kernel_spmd

B = 512
D = 256
C = 100000
NCORES = 8
CSH = C // NCORES          # 12500 classes per core
# class tiles per core: six 2048-wide + one 256-wide = 12544.
# Most batch chunks put the 256-wide tile first (cheap pipeline starter);
# the last chunk puts it last so the wide columns can reduce early.
TILES_SMALL_FIRST = [(12288, 256)] + [(i * 2048, 2048) for i in range(6)]
TILES_SMALL_LAST = [(i * 2048, 2048) for i in range(6)] + [(12288, 256)]
CPAD = 12544
NPAD_TOT = (CPAD - CSH) * NCORES

M0 = 0.5
M_MIN = 0.25
SCALE = 32.0
SHIFT = 32.0               # fixed log-softmax shift (|logits| <= SCALE)
FP8_PRESCALE = 8.0         # both operands scaled by 8 -> matmul gives 64*cos

f32 = mybir.dt.float32
bf16 = mybir.dt.bfloat16
fp8 = mybir.dt.float8e4

NBC = B // 128             # 4 batch chunks

_cached_nc = None
_last_results = None


def _build():
    global _cached_nc
    if _cached_nc is not None:
        return _cached_nc

    nc = bacc.Bacc(
        "TRN2", target_bir_lowering=False, debug=False, num_devices=NCORES
    )

    # [p, j, c] with contraction index k = j*128 + p
    wnT_d = nc.dram_tensor("wnT", [128, 2, CPAD], fp8, kind="ExternalInput")
    featnT_d = nc.dram_tensor("featnT", [128, 2, B], fp8, kind="ExternalInput")
    out_d = nc.dram_tensor("out", [128, NBC], f32, kind="ExternalOutput")

    with tile.TileContext(nc) as tc:
        with (
            tc.tile_pool(name="persist", bufs=1) as persist,
            tc.tile_pool(name="epool", bufs=3) as epool,
            tc.tile_pool(name="psum", bufs=2, space="PSUM") as psum,
        ):
            fsb = persist.tile([128, 2, B], fp8)
            nc.sync.dma_start(out=fsb[:], in_=featnT_d[:])

            wsb = persist.tile([128, 2, CPAD], fp8)
            # chunked loads in consumption order: the small tile's columns
            # first (gates the pipeline start), then the wide region.
            # Spread across the three DMA-capable engines' queues — a single
            # queue serializes at ~50GB/s and trickles for the whole kernel.
            # (engine, lo, hi) in consumption order across the two HWDGE
            # queues (gpsimd's SWDGE path is too slow for bulk loads)
            plan = [
                (nc.sync, 12288, 12544),
                (nc.sync, 0, 1536),
                (nc.scalar, 1536, 3072),
                (nc.sync, 3072, 4608),
                (nc.scalar, 4608, 6144),
                (nc.sync, 6144, 7680),
                (nc.scalar, 7680, 9216),
                (nc.sync, 9216, 10752),
                (nc.scalar, 10752, 12288),
            ]
            for eng, lo, hi in plan:
                eng.dma_start(
                    out=wsb[:, :, lo:hi], in_=wnT_d[:, :, lo:hi]
                )

            bias_s = persist.tile([128, 1], f32)
            nc.gpsimd.memset(bias_s[:], -SHIFT)

            eacc = [
                persist.tile(
                    [128, 2048], bf16, tag=f"eacc{bc}", name=f"eacc{bc}"
                )
                for bc in range(NBC)
            ]

            S_all = persist.tile([128, NBC], f32)

            S_main = persist.tile([128, 1], f32)

            for bc in range(NBC):
                last = bc == NBC - 1
                tiles = TILES_SMALL_LAST if last else TILES_SMALL_FIRST
                lhs = fsb[:, :, bc * 128:(bc + 1) * 128]
                for ti, (c0, cw) in enumerate(tiles):
                    ps = psum.tile([128, 2048], f32, tag="ps")
                    for j in range(0, cw, 512):
                        jw = min(512, cw - j)
                        nc.tensor.matmul(
                            ps[:, j:j + jw],
                            lhs,
                            wsb[:, :, c0 + j:c0 + j + jw],
                            start=True, stop=True,
                            perf_mode=mybir.MatmulPerfMode.DoubleRow,
                        )
                    esc = epool.tile([128, 2048], bf16, tag="esc")
                    nc.scalar.activation(
                        esc[:, :cw], ps[:, :cw],
                        mybir.ActivationFunctionType.Exp,
                        bias=bias_s[:], scale=SCALE / (FP8_PRESCALE**2),
                    )
                    if ti == 0:
                        nc.vector.tensor_copy(
                            eacc[bc][:, :cw], esc[:, :cw]
                        )
                    elif ti == 1 and not last:
                        nc.vector.tensor_add(
                            eacc[bc][:, :256], eacc[bc][:, :256], esc[:, :256]
                        )
                        nc.vector.tensor_copy(
                            eacc[bc][:, 256:], esc[:, 256:]
                        )
                    else:
                        nc.vector.tensor_add(
                            eacc[bc][:, :cw], eacc[bc][:, :cw], esc[:, :cw]
                        )
                    if bc > 0 and ti == 1:
                        nc.vector.tensor_reduce(
                            S_all[:, bc - 1:bc],
                            eacc[bc - 1][:],
                            axis=mybir.AxisListType.X,
                            op=mybir.AluOpType.add,
                        )
                    if last and ti == len(tiles) - 2:
                        nc.vector.tensor_reduce(
                            S_main[:],
                            eacc[bc][:, 256:2048],
                            axis=mybir.AxisListType.X,
                            op=mybir.AluOpType.add,
                        )

            S_small = persist.tile([128, 1], f32)
            nc.vector.tensor_reduce(
                S_small[:],
                eacc[NBC - 1][:, 0:256],
                axis=mybir.AxisListType.X,
                op=mybir.AluOpType.add,
            )
            nc.vector.tensor_add(S_all[:, NBC - 1:NBC], S_main[:], S_small[:])

            nc.sync.dma_start(out=out_d[:], in_=S_all[:])

    nc.compile()
    _cached_nc = nc
    return nc


def _host_prep(features, weight, weights, labels):
    """Everything O(B*D) / O(C*D) that is not the big matmul."""
    f = features.astype(np.float64)
    norms = np.sqrt((f * f).sum(axis=1))
    lo, hi = norms.min(), norms.max()
    denom = max(hi - lo, 1e-8)
    margins = np.clip(M_MIN + (M0 - M_MIN) * (norms - lo) / denom, M_MIN, M0)
    feat_n = f / np.maximum(norms, 1e-12)[:, None]

    wlab = weight[labels].astype(np.float64)
    wlab_n = wlab / np.maximum(
        np.sqrt((wlab * wlab).sum(axis=1)), 1e-12
    )[:, None]
    cos_t = np.clip((feat_n * wlab_n).sum(axis=1), -1.0 + 1e-7, 1.0 - 1e-7)
    cos_m = cos_t * np.cos(margins) - np.sqrt(1.0 - cos_t * cos_t) * np.sin(
        margins
    )
    t_logit = SCALE * cos_m
    corr = (
        np.exp(SCALE * cos_m - SHIFT)
        - np.exp(SCALE * cos_t - SHIFT)
        - NPAD_TOT * np.exp(-SHIFT)
    )
    coef = weights.astype(np.float64) / B
    return feat_n, corr, coef, t_logit


def _to_dr_layout(mat_t, width):
    """[D, X] f32 -> [128, 2, X] fp8 with k = j*128 + p."""
    a = mat_t.reshape(2, 128, width)          # [j, p, X]
    a = np.ascontiguousarray(a.transpose(1, 0, 2))  # [p, j, X]
    return a.astype(ml_dtypes.float8_e4m3)


def kernel(features, weight, weights, labels):
    global _last_results
    features = np.asarray(features, dtype=np.float32)
    weight = np.asarray(weight, dtype=np.float32)
    weights = np.asarray(weights, dtype=np.float32)
    labels = np.asarray(labels).astype(np.int64)

    feat_n, corr, coef, t_logit = _host_prep(features, weight, weights, labels)

    wn = weight / np.maximum(
        np.linalg.norm(weight, axis=1, keepdims=True), 1e-12
    )
    featnT = np.ascontiguousarray(feat_n.T.astype(np.float32)) * FP8_PRESCALE
    featnT8 = _to_dr_layout(featnT, B)

    in_maps = []
    for i in range(NCORES):
        sh = wn[i * CSH:(i + 1) * CSH]  # [CSH, D]
        wt = np.zeros((D, CPAD), dtype=np.float32)
        wt[:, :CSH] = sh.T * FP8_PRESCALE
        in_maps.append(
            {"wnT": _to_dr_layout(wt, CPAD), "featnT": featnT8}
        )

    nc = _build()
    res = run_bass_kernel_spmd(nc, in_maps, list(range(NCORES)))
    _last_results = res

    # ---- host combine ----
    S = np.zeros(B, dtype=np.float64)
    for i in range(NCORES):
        sc = np.asarray(res.results[i]["out"], dtype=np.float64)  # [128, 4]
        for bc in range(NBC):
            S[bc * 128:(bc + 1) * 128] += sc[:, bc]

    Z = S + corr
    per = SHIFT + np.log(Z) - t_logit
    loss = float((coef * per).sum())
    return np.array(loss, dtype=np.float32)



# revision 39
# speedup vs baseline: 1.1660x; 1.0210x over previous
"""AdaFace loss on 8 TRN2 NeuronCores, class-parallel.

Strategy: shard the 100k weight rows (classes) across 8 cores. Host
pre-normalizes rows, transposes to [D, C_shard], scales by 8 and casts to
fp8e4 (scale keeps values out of the e4m3 subnormal range; the ScalarE
exp absorbs it: exp(0.5*x - 32) of the 64*cos matmul result). Since
|logit| <= 32, a fixed shift of 32 replaces the per-row max of a
standard log-softmax, so no max collective is needed. Each core returns
per-batch-chunk partial sums of exp(32c-32); the host does the final
O(B) combine: sum across cores, margin-target correction (cos(theta+m)
needs only sqrt, no arccos), ln, weighted dot. No device collective.

Device per core: the whole fp8 weight shard stays resident in SBUF
(24.5KB/partition); DoubleRow matmuls (K=256 per instruction) fill
4-bank PSUM tiles [128b, 2048c]; ScalarE exp -> bf16; VectorE 2x-mode
running adds + one final reduce per batch chunk; single DMA out.
"""

import numpy as np
import ml_dtypes

import concourse.bass as bass
import concourse.tile as tile
from concourse import bacc, mybir
from concourse.bass_utils import run_bass_kernel_spmd

B = 512
D = 256
C = 100000
NCORES = 8
CSH = C // NCORES          # 12500 classes per core
# class tiles per core: six 2048-wide + one 256-wide = 12544.
# Most batch chunks put the 256-wide tile first (cheap pipeline starter);
# the last chunk puts it last so the wide columns can reduce early.
TILES_SMALL_FIRST = [(12288, 256)] + [(i * 2048, 2048) for i in range(6)]
TILES_SMALL_LAST = [(i * 2048, 2048) for i in range(6)] + [(12288, 256)]
CPAD = 12544
NPAD_TOT = (CPAD - CSH) * NCORES

M0 = 0.5
M_MIN = 0.25
SCALE = 32.0
SHIFT = 32.0               # fixed log-softmax shift (|logits| <= SCALE)
FP8_PRESCALE = 8.0         # both operands scaled by 8 -> matmul gives 64*cos

f32 = mybir.dt.float32
bf16 = mybir.dt.bfloat16
fp8 = mybir.dt.float8e4

NBC = B // 128             # 4 batch chunks

_cached_nc = None
_last_results = None


def _build():
    global _cached_nc
    if _cached_nc is not None:
        return _cached_nc

    nc = bacc.Bacc(
        "TRN2", target_bir_lowering=False, debug=False, num_devices=NCORES
    )

    # [p, j, c] with contraction index k = j*128 + p
    wnT_d = nc.dram_tensor("wnT", [128, 2, CPAD], fp8, kind="ExternalInput")
    featnT_d = nc.dram_tensor("featnT", [128, 2, B], fp8, kind="ExternalInput")
    out_d = nc.dram_tensor("out", [128, NBC], f32, kind="ExternalOutput")

    with tile.TileContext(nc) as tc:
        with (
            tc.tile_pool(name="persist", bufs=1) as persist,
            tc.tile_pool(name="epool", bufs=3) as epool,
            tc.tile_pool(name="psum", bufs=2, space="PSUM") as psum,
        ):
            fsb = persist.tile([128, 2, B], fp8)
            nc.sync.dma_start(out=fsb[:], in_=featnT_d[:])

            wsb = persist.tile([128, 2, CPAD], fp8)
            # chunked loads in consumption order: the small tile's columns
            # first (gates the pipeline start), then the wide region.
            # Spread across the three DMA-capable engines' queues — a single
            # queue serializes at ~50GB/s and trickles for the whole kernel.
            # (engine, lo, hi) in consumption order across the two HWDGE
            # queues (gpsimd's SWDGE path is too slow for bulk loads)
            plan = [
                (nc.scalar, 12288, 12544),
                (nc.scalar, 0, 1536),
                (nc.sync, 1536, 3072),
                (nc.scalar, 3072, 4608),
                (nc.sync, 4608, 6144),
                (nc.scalar, 6144, 7680),
                (nc.sync, 7680, 9216),
                (nc.scalar, 9216, 10752),
                (nc.sync, 10752, 12288),
            ]
            for eng, lo, hi in plan:
                eng.dma_start(
                    out=wsb[:, :, lo:hi], in_=wnT_d[:, :, lo:hi]
                )

            bias_s = persist.tile([128, 1], f32)
            nc.gpsimd.memset(bias_s[:], -SHIFT)

            eacc = [
                persist.tile(
                    [128, 2048], bf16, tag=f"eacc{bc}", name=f"eacc{bc}"
                )
                for bc in range(NBC)
            ]

            S_all = persist.tile([128, NBC], f32)

            S_main = persist.tile([128, 1], f32)

            for bc in range(NBC):
                last = bc == NBC - 1
                tiles = TILES_SMALL_LAST if last else TILES_SMALL_FIRST
                lhs = fsb[:, :, bc * 128:(bc + 1) * 128]
                for ti, (c0, cw) in enumerate(tiles):
                    ps = psum.tile([128, 2048], f32, tag="ps")
                    for j in range(0, cw, 512):
                        jw = min(512, cw - j)
                        nc.tensor.matmul(
                            ps[:, j:j + jw],
                            lhs,
                            wsb[:, :, c0 + j:c0 + j + jw],
                            start=True, stop=True,
                            perf_mode=mybir.MatmulPerfMode.DoubleRow,
                        )
                    esc = epool.tile([128, 2048], bf16, tag="esc")
                    nc.scalar.activation(
                        esc[:, :cw], ps[:, :cw],
                        mybir.ActivationFunctionType.Exp,
                        bias=bias_s[:], scale=SCALE / (FP8_PRESCALE**2),
                    )
                    if ti == 0:
                        nc.vector.tensor_copy(
                            eacc[bc][:, :cw], esc[:, :cw]
                        )
                    elif ti == 1 and not last:
                        nc.vector.tensor_add(
                            eacc[bc][:, :256], eacc[bc][:, :256], esc[:, :256]
                        )
                        nc.vector.tensor_copy(
                            eacc[bc][:, 256:], esc[:, 256:]
                        )
                    else:
                        nc.vector.tensor_add(
                            eacc[bc][:, :cw], eacc[bc][:, :cw], esc[:, :cw]
                        )
                    if bc > 0 and ti == 1:
                        nc.vector.tensor_reduce(
                            S_all[:, bc - 1:bc],
                            eacc[bc - 1][:],
                            axis=mybir.AxisListType.X,
                            op=mybir.AluOpType.add,
                        )
                    if last and ti == len(tiles) - 2:
                        nc.vector.tensor_reduce(
                            S_main[:],
                            eacc[bc][:, 256:2048],
                            axis=mybir.AxisListType.X,
                            op=mybir.AluOpType.add,
                        )

            S_small = persist.tile([128, 1], f32)
            nc.vector.tensor_reduce(
                S_small[:],
                eacc[NBC - 1][:, 0:256],
                axis=mybir.AxisListType.X,
                op=mybir.AluOpType.add,
            )
            nc.vector.tensor_add(S_all[:, NBC - 1:NBC], S_main[:], S_small[:])

            nc.sync.dma_start(out=out_d[:], in_=S_all[:])

    nc.compile()
    _cached_nc = nc
    return nc


def _host_prep(features, weight, weights, labels):
    """Everything O(B*D) / O(C*D) that is not the big matmul."""
    f = features.astype(np.float64)
    norms = np.sqrt((f * f).sum(axis=1))
    lo, hi = norms.min(), norms.max()
    denom = max(hi - lo, 1e-8)
    margins = np.clip(M_MIN + (M0 - M_MIN) * (norms - lo) / denom, M_MIN, M0)
    feat_n = f / np.maximum(norms, 1e-12)[:, None]

    wlab = weight[labels].astype(np.float64)
    wlab_n = wlab / np.maximum(
        np.sqrt((wlab * wlab).sum(axis=1)), 1e-12
    )[:, None]
    cos_t = np.clip((feat_n * wlab_n).sum(axis=1), -1.0 + 1e-7, 1.0 - 1e-7)
    cos_m = cos_t * np.cos(margins) - np.sqrt(1.0 - cos_t * cos_t) * np.sin(
        margins
    )
    t_logit = SCALE * cos_m
    corr = (
        np.exp(SCALE * cos_m - SHIFT)
        - np.exp(SCALE * cos_t - SHIFT)
        - NPAD_TOT * np.exp(-SHIFT)
    )
    coef = weights.astype(np.float64) / B
    return feat_n, corr, coef, t_logit


def _to_dr_layout(mat_t, width):
    """[D, X] f32 -> [128, 2, X] fp8 with k = j*128 + p."""
    a = mat_t.reshape(2, 128, width)          # [j, p, X]
    a = np.ascontiguousarray(a.transpose(1, 0, 2))  # [p, j, X]
    return a.astype(ml_dtypes.float8_e4m3)


def kernel(features, weight, weights, labels):
    global _last_results
    features = np.asarray(features, dtype=np.float32)
    weight = np.asarray(weight, dtype=np.float32)
    weights = np.asarray(weights, dtype=np.float32)
    labels = np.asarray(labels).astype(np.int64)

    feat_n, corr, coef, t_logit = _host_prep(features, weight, weights, labels)

    wn = weight / np.maximum(
        np.linalg.norm(weight, axis=1, keepdims=True), 1e-12
    )
    featnT = np.ascontiguousarray(feat_n.T.astype(np.float32)) * FP8_PRESCALE
    featnT8 = _to_dr_layout(featnT, B)

    in_maps = []
    for i in range(NCORES):
        sh = wn[i * CSH:(i + 1) * CSH]  # [CSH, D]
        wt = np.zeros((D, CPAD), dtype=np.float32)
        wt[:, :CSH] = sh.T * FP8_PRESCALE
        in_maps.append(
            {"wnT": _to_dr_layout(wt, CPAD), "featnT": featnT8}
        )

    nc = _build()
    res = run_bass_kernel_spmd(nc, in_maps, list(range(NCORES)))
    _last_results = res

    # ---- host combine ----
    S = np.zeros(B, dtype=np.float64)
    for i in range(NCORES):
        sc = np.asarray(res.results[i]["out"], dtype=np.float64)  # [128, 4]
        for bc in range(NBC):
            S[bc * 128:(bc + 1) * 128] += sc[:, bc]

    Z = S + corr
    per = SHIFT + np.log(Z) - t_logit
    loss = float((coef * per).sum())
    return np.array(loss, dtype=np.float32)



# revision 40
# speedup vs baseline: 1.2020x; 1.0309x over previous
"""AdaFace loss on 8 TRN2 NeuronCores, class-parallel.

Strategy: shard the 100k weight rows (classes) across 8 cores. Host
pre-normalizes rows, transposes to [D, C_shard], scales by 8 and casts to
fp8e4 (scale keeps values out of the e4m3 subnormal range; the ScalarE
exp absorbs it: exp(0.5*x - 32) of the 64*cos matmul result). Since
|logit| <= 32, a fixed shift of 32 replaces the per-row max of a
standard log-softmax, so no max collective is needed. Each core returns
per-batch-chunk partial sums of exp(32c-32); the host does the final
O(B) combine: sum across cores, margin-target correction (cos(theta+m)
needs only sqrt, no arccos), ln, weighted dot. No device collective.

Device per core: the whole fp8 weight shard stays resident in SBUF
(24.5KB/partition); DoubleRow matmuls (K=256 per instruction) fill
4-bank PSUM tiles [128b, 2048c]; ScalarE exp -> bf16; VectorE 2x-mode
running adds + one final reduce per batch chunk; single DMA out.
"""

import numpy as np
import ml_dtypes

import concourse.bass as bass
import concourse.tile as tile
from concourse import bacc, mybir
from concourse.bass_utils import run_bass_kernel_spmd

B = 512
D = 256
C = 100000
NCORES = 8
CSH = C // NCORES          # 12500 classes per core
# class tiles per core: six 2048-wide + one 256-wide = 12544.
# Most batch chunks put the 256-wide tile first (cheap pipeline starter);
# the last chunk puts it last so the wide columns can reduce early.
TILES_SMALL_FIRST = [(12288, 256)] + [(i * 2048, 2048) for i in range(6)]
TILES_SMALL_LAST = [(i * 2048, 2048) for i in range(6)] + [(12288, 256)]
CPAD = 12544
NPAD_TOT = (CPAD - CSH) * NCORES

M0 = 0.5
M_MIN = 0.25
SCALE = 32.0
SHIFT = 32.0               # fixed log-softmax shift (|logits| <= SCALE)
FP8_PRESCALE = 8.0         # both operands scaled by 8 -> matmul gives 64*cos

f32 = mybir.dt.float32
bf16 = mybir.dt.bfloat16
fp8 = mybir.dt.float8e4

NBC = B // 128             # 4 batch chunks

_cached_nc = None
_last_results = None


def _build():
    global _cached_nc
    if _cached_nc is not None:
        return _cached_nc

    nc = bacc.Bacc(
        "TRN2", target_bir_lowering=False, debug=False, num_devices=NCORES
    )

    # [p, j, c] with contraction index k = j*128 + p
    wnT_d = nc.dram_tensor("wnT", [128, 2, CPAD], fp8, kind="ExternalInput")
    featnT_d = nc.dram_tensor("featnT", [128, 2, B], fp8, kind="ExternalInput")
    out_d = nc.dram_tensor("out", [128, NBC], f32, kind="ExternalOutput")

    with tile.TileContext(nc) as tc:
        with (
            tc.tile_pool(name="persist", bufs=1) as persist,
            tc.tile_pool(name="epool", bufs=5) as epool,
            tc.tile_pool(name="psum", bufs=2, space="PSUM") as psum,
        ):
            fsb = persist.tile([128, 2, B], fp8)
            nc.sync.dma_start(out=fsb[:], in_=featnT_d[:])

            wsb = persist.tile([128, 2, CPAD], fp8)
            # chunked loads in consumption order: the small tile's columns
            # first (gates the pipeline start), then the wide region.
            # Spread across the three DMA-capable engines' queues — a single
            # queue serializes at ~50GB/s and trickles for the whole kernel.
            # (engine, lo, hi) in consumption order across the two HWDGE
            # queues (gpsimd's SWDGE path is too slow for bulk loads)
            plan = [
                (nc.scalar, 12288, 12544),
                (nc.scalar, 0, 1536),
                (nc.sync, 1536, 3072),
                (nc.scalar, 3072, 4608),
                (nc.sync, 4608, 6144),
                (nc.scalar, 6144, 7680),
                (nc.sync, 7680, 9216),
                (nc.scalar, 9216, 10752),
                (nc.sync, 10752, 12288),
            ]
            for eng, lo, hi in plan:
                eng.dma_start(
                    out=wsb[:, :, lo:hi], in_=wnT_d[:, :, lo:hi]
                )

            bias_s = persist.tile([128, 1], f32)
            nc.gpsimd.memset(bias_s[:], -SHIFT)

            eacc = [
                persist.tile(
                    [128, 2048], bf16, tag=f"eacc{bc}", name=f"eacc{bc}"
                )
                for bc in range(NBC)
            ]

            S_all = persist.tile([128, NBC], f32)

            S_main = persist.tile([128, 1], f32)

            for bc in range(NBC):
                last = bc == NBC - 1
                tiles = TILES_SMALL_LAST if last else TILES_SMALL_FIRST
                lhs = fsb[:, :, bc * 128:(bc + 1) * 128]
                for ti, (c0, cw) in enumerate(tiles):
                    ps = psum.tile([128, 2048], f32, tag="ps")
                    for j in range(0, cw, 512):
                        jw = min(512, cw - j)
                        nc.tensor.matmul(
                            ps[:, j:j + jw],
                            lhs,
                            wsb[:, :, c0 + j:c0 + j + jw],
                            start=True, stop=True,
                            perf_mode=mybir.MatmulPerfMode.DoubleRow,
                        )
                    esc = epool.tile([128, 2048], bf16, tag="esc")
                    nc.scalar.activation(
                        esc[:, :cw], ps[:, :cw],
                        mybir.ActivationFunctionType.Exp,
                        bias=bias_s[:], scale=SCALE / (FP8_PRESCALE**2),
                    )
                    if ti == 0:
                        nc.vector.tensor_copy(
                            eacc[bc][:, :cw], esc[:, :cw]
                        )
                    elif ti == 1 and not last:
                        nc.vector.tensor_add(
                            eacc[bc][:, :256], eacc[bc][:, :256], esc[:, :256]
                        )
                        nc.vector.tensor_copy(
                            eacc[bc][:, 256:], esc[:, 256:]
                        )
                    else:
                        nc.vector.tensor_add(
                            eacc[bc][:, :cw], eacc[bc][:, :cw], esc[:, :cw]
                        )
                    if bc > 0 and ti == 1:
                        nc.vector.tensor_reduce(
                            S_all[:, bc - 1:bc],
                            eacc[bc - 1][:],
                            axis=mybir.AxisListType.X,
                            op=mybir.AluOpType.add,
                        )
                    if last and ti == len(tiles) - 2:
                        nc.vector.tensor_reduce(
                            S_main[:],
                            eacc[bc][:, 256:2048],
                            axis=mybir.AxisListType.X,
                            op=mybir.AluOpType.add,
                        )

            S_small = persist.tile([128, 1], f32)
            nc.vector.tensor_reduce(
                S_small[:],
                eacc[NBC - 1][:, 0:256],
                axis=mybir.AxisListType.X,
                op=mybir.AluOpType.add,
            )
            nc.vector.tensor_add(S_all[:, NBC - 1:NBC], S_main[:], S_small[:])

            nc.sync.dma_start(out=out_d[:], in_=S_all[:])

    nc.compile()
    _cached_nc = nc
    return nc


def _host_prep(features, weight, weights, labels):
    """Everything O(B*D) / O(C*D) that is not the big matmul."""
    f = features.astype(np.float64)
    norms = np.sqrt((f * f).sum(axis=1))
    lo, hi = norms.min(), norms.max()
    denom = max(hi - lo, 1e-8)
    margins = np.clip(M_MIN + (M0 - M_MIN) * (norms - lo) / denom, M_MIN, M0)
    feat_n = f / np.maximum(norms, 1e-12)[:, None]

    wlab = weight[labels].astype(np.float64)
    wlab_n = wlab / np.maximum(
        np.sqrt((wlab * wlab).sum(axis=1)), 1e-12
    )[:, None]
    cos_t = np.clip((feat_n * wlab_n).sum(axis=1), -1.0 + 1e-7, 1.0 - 1e-7)
    cos_m = cos_t * np.cos(margins) - np.sqrt(1.0 - cos_t * cos_t) * np.sin(
        margins
    )
    t_logit = SCALE * cos_m
    corr = (
        np.exp(SCALE * cos_m - SHIFT)
        - np.exp(SCALE * cos_t - SHIFT)
        - NPAD_TOT * np.exp(-SHIFT)
    )
    coef = weights.astype(np.float64) / B
    return feat_n, corr, coef, t_logit


def _to_dr_layout(mat_t, width):
    """[D, X] f32 -> [128, 2, X] fp8 with k = j*128 + p."""
    a = mat_t.reshape(2, 128, width)          # [j, p, X]
    a = np.ascontiguousarray(a.transpose(1, 0, 2))  # [p, j, X]
    return a.astype(ml_dtypes.float8_e4m3)


def kernel(features, weight, weights, labels):
    global _last_results
    features = np.asarray(features, dtype=np.float32)
    weight = np.asarray(weight, dtype=np.float32)
    weights = np.asarray(weights, dtype=np.float32)
    labels = np.asarray(labels).astype(np.int64)

    feat_n, corr, coef, t_logit = _host_prep(features, weight, weights, labels)

    wn = weight / np.maximum(
        np.linalg.norm(weight, axis=1, keepdims=True), 1e-12
    )
    featnT = np.ascontiguousarray(feat_n.T.astype(np.float32)) * FP8_PRESCALE
    featnT8 = _to_dr_layout(featnT, B)

    in_maps = []
    for i in range(NCORES):
        sh = wn[i * CSH:(i + 1) * CSH]  # [CSH, D]
        wt = np.zeros((D, CPAD), dtype=np.float32)
        wt[:, :CSH] = sh.T * FP8_PRESCALE
        in_maps.append(
            {"wnT": _to_dr_layout(wt, CPAD), "featnT": featnT8}
        )

    nc = _build()
    res = run_bass_kernel_spmd(nc, in_maps, list(range(NCORES)))
    _last_results = res

    # ---- host combine ----
    S = np.zeros(B, dtype=np.float64)
    for i in range(NCORES):
        sc = np.asarray(res.results[i]["out"], dtype=np.float64)  # [128, 4]
        for bc in range(NBC):
            S[bc * 128:(bc + 1) * 128] += sc[:, bc]

    Z = S + corr
    per = SHIFT + np.log(Z) - t_logit
    loss = float((coef * per).sum())
    return np.array(loss, dtype=np.float32)



# revision 41
# speedup vs baseline: 1.2027x; 1.0006x over previous
"""AdaFace loss on 8 TRN2 NeuronCores, class-parallel.

Strategy: shard the 100k weight rows (classes) across 8 cores. Host
pre-normalizes rows, transposes to [D, C_shard], scales by 8 and casts to
fp8e4 (scale keeps values out of the e4m3 subnormal range; the ScalarE
exp absorbs it: exp(0.5*x - 32) of the 64*cos matmul result). Since
|logit| <= 32, a fixed shift of 32 replaces the per-row max of a
standard log-softmax, so no max collective is needed. Each core returns
per-batch-chunk partial sums of exp(32c-32); the host does the final
O(B) combine: sum across cores, margin-target correction (cos(theta+m)
needs only sqrt, no arccos), ln, weighted dot. No device collective.

Device per core: the whole fp8 weight shard stays resident in SBUF
(24.5KB/partition); DoubleRow matmuls (K=256 per instruction) fill
4-bank PSUM tiles [128b, 2048c]; ScalarE exp -> bf16; VectorE 2x-mode
running adds + one final reduce per batch chunk; single DMA out.
"""

import numpy as np
import ml_dtypes

import concourse.bass as bass
import concourse.tile as tile
from concourse import bacc, mybir
from concourse.bass_utils import run_bass_kernel_spmd

B = 512
D = 256
C = 100000
NCORES = 8
CSH = C // NCORES          # 12500 classes per core
# class tiles per core: six 2048-wide + one 256-wide = 12544.
# Most batch chunks put the 256-wide tile first (cheap pipeline starter);
# the last chunk puts it last so the wide columns can reduce early.
TILES_SMALL_FIRST = [(12288, 256)] + [(i * 2048, 2048) for i in range(6)]
TILES_SMALL_LAST = [(i * 2048, 2048) for i in range(6)] + [(12288, 256)]
CPAD = 12544
NPAD_TOT = (CPAD - CSH) * NCORES

M0 = 0.5
M_MIN = 0.25
SCALE = 32.0
SHIFT = 32.0               # fixed log-softmax shift (|logits| <= SCALE)
FP8_PRESCALE = 8.0         # both operands scaled by 8 -> matmul gives 64*cos

f32 = mybir.dt.float32
bf16 = mybir.dt.bfloat16
fp8 = mybir.dt.float8e4

NBC = B // 128             # 4 batch chunks

_cached_nc = None
_last_results = None


def _build():
    global _cached_nc
    if _cached_nc is not None:
        return _cached_nc

    nc = bacc.Bacc(
        "TRN2", target_bir_lowering=False, debug=False, num_devices=NCORES
    )

    # [p, j, c] with contraction index k = j*128 + p
    wnT_d = nc.dram_tensor("wnT", [128, 2, CPAD], fp8, kind="ExternalInput")
    featnT_d = nc.dram_tensor("featnT", [128, 2, B], fp8, kind="ExternalInput")
    out_d = nc.dram_tensor("out", [128, NBC], f32, kind="ExternalOutput")

    with tile.TileContext(nc) as tc:
        with (
            tc.tile_pool(name="persist", bufs=1) as persist,
            tc.tile_pool(name="epool", bufs=6) as epool,
            tc.tile_pool(name="psum", bufs=2, space="PSUM") as psum,
        ):
            fsb = persist.tile([128, 2, B], fp8)
            nc.sync.dma_start(out=fsb[:], in_=featnT_d[:])

            wsb = persist.tile([128, 2, CPAD], fp8)
            # chunked loads in consumption order: the small tile's columns
            # first (gates the pipeline start), then the wide region.
            # Spread across the three DMA-capable engines' queues — a single
            # queue serializes at ~50GB/s and trickles for the whole kernel.
            # (engine, lo, hi) in consumption order across the two HWDGE
            # queues (gpsimd's SWDGE path is too slow for bulk loads)
            plan = [
                (nc.scalar, 12288, 12544),
                (nc.scalar, 0, 1536),
                (nc.sync, 1536, 3072),
                (nc.scalar, 3072, 4608),
                (nc.sync, 4608, 6144),
                (nc.scalar, 6144, 7680),
                (nc.sync, 7680, 9216),
                (nc.scalar, 9216, 10752),
                (nc.sync, 10752, 12288),
            ]
            for eng, lo, hi in plan:
                eng.dma_start(
                    out=wsb[:, :, lo:hi], in_=wnT_d[:, :, lo:hi]
                )

            bias_s = persist.tile([128, 1], f32)
            nc.gpsimd.memset(bias_s[:], -SHIFT)

            eacc = [
                persist.tile(
                    [128, 2048], bf16, tag=f"eacc{bc}", name=f"eacc{bc}"
                )
                for bc in range(NBC)
            ]

            S_all = persist.tile([128, NBC], f32)

            S_main = persist.tile([128, 1], f32)

            for bc in range(NBC):
                last = bc == NBC - 1
                tiles = TILES_SMALL_LAST if last else TILES_SMALL_FIRST
                lhs = fsb[:, :, bc * 128:(bc + 1) * 128]
                for ti, (c0, cw) in enumerate(tiles):
                    ps = psum.tile([128, 2048], f32, tag="ps")
                    for j in range(0, cw, 512):
                        jw = min(512, cw - j)
                        nc.tensor.matmul(
                            ps[:, j:j + jw],
                            lhs,
                            wsb[:, :, c0 + j:c0 + j + jw],
                            start=True, stop=True,
                            perf_mode=mybir.MatmulPerfMode.DoubleRow,
                        )
                    esc = epool.tile([128, 2048], bf16, tag="esc")
                    nc.scalar.activation(
                        esc[:, :cw], ps[:, :cw],
                        mybir.ActivationFunctionType.Exp,
                        bias=bias_s[:], scale=SCALE / (FP8_PRESCALE**2),
                    )
                    if ti == 0:
                        nc.vector.tensor_copy(
                            eacc[bc][:, :cw], esc[:, :cw]
                        )
                    elif ti == 1 and not last:
                        nc.vector.tensor_add(
                            eacc[bc][:, :256], eacc[bc][:, :256], esc[:, :256]
                        )
                        nc.vector.tensor_copy(
                            eacc[bc][:, 256:], esc[:, 256:]
                        )
                    else:
                        nc.vector.tensor_add(
                            eacc[bc][:, :cw], eacc[bc][:, :cw], esc[:, :cw]
                        )
                    if bc > 0 and ti == 1:
                        nc.vector.tensor_reduce(
                            S_all[:, bc - 1:bc],
                            eacc[bc - 1][:],
                            axis=mybir.AxisListType.X,
                            op=mybir.AluOpType.add,
                        )
                    if last and ti == len(tiles) - 2:
                        nc.vector.tensor_reduce(
                            S_main[:],
                            eacc[bc][:, 256:2048],
                            axis=mybir.AxisListType.X,
                            op=mybir.AluOpType.add,
                        )

            S_small = persist.tile([128, 1], f32)
            nc.vector.tensor_reduce(
                S_small[:],
                eacc[NBC - 1][:, 0:256],
                axis=mybir.AxisListType.X,
                op=mybir.AluOpType.add,
            )
            nc.vector.tensor_add(S_all[:, NBC - 1:NBC], S_main[:], S_small[:])

            nc.sync.dma_start(out=out_d[:], in_=S_all[:])

    nc.compile()
    _cached_nc = nc
    return nc


def _host_prep(features, weight, weights, labels):
    """Everything O(B*D) / O(C*D) that is not the big matmul."""
    f = features.astype(np.float64)
    norms = np.sqrt((f * f).sum(axis=1))
    lo, hi = norms.min(), norms.max()
    denom = max(hi - lo, 1e-8)
    margins = np.clip(M_MIN + (M0 - M_MIN) * (norms - lo) / denom, M_MIN, M0)
    feat_n = f / np.maximum(norms, 1e-12)[:, None]

    wlab = weight[labels].astype(np.float64)
    wlab_n = wlab / np.maximum(
        np.sqrt((wlab * wlab).sum(axis=1)), 1e-12
    )[:, None]
    cos_t = np.clip((feat_n * wlab_n).sum(axis=1), -1.0 + 1e-7, 1.0 - 1e-7)
    cos_m = cos_t * np.cos(margins) - np.sqrt(1.0 - cos_t * cos_t) * np.sin(
        margins
    )
    t_logit = SCALE * cos_m
    corr = (
        np.exp(SCALE * cos_m - SHIFT)
        - np.exp(SCALE * cos_t - SHIFT)
        - NPAD_TOT * np.exp(-SHIFT)
    )
    coef = weights.astype(np.float64) / B
    return feat_n, corr, coef, t_logit


def _to_dr_layout(mat_t, width):
    """[D, X] f32 -> [128, 2, X] fp8 with k = j*128 + p."""
    a = mat_t.reshape(2, 128, width)          # [j, p, X]
    a = np.ascontiguousarray(a.transpose(1, 0, 2))  # [p, j, X]
    return a.astype(ml_dtypes.float8_e4m3)


def kernel(features, weight, weights, labels):
    global _last_results
    features = np.asarray(features, dtype=np.float32)
    weight = np.asarray(weight, dtype=np.float32)
    weights = np.asarray(weights, dtype=np.float32)
    labels = np.asarray(labels).astype(np.int64)

    feat_n, corr, coef, t_logit = _host_prep(features, weight, weights, labels)

    wn = weight / np.maximum(
        np.linalg.norm(weight, axis=1, keepdims=True), 1e-12
    )
    featnT = np.ascontiguousarray(feat_n.T.astype(np.float32)) * FP8_PRESCALE
    featnT8 = _to_dr_layout(featnT, B)

    in_maps = []
    for i in range(NCORES):
        sh = wn[i * CSH:(i + 1) * CSH]  # [CSH, D]
        wt = np.zeros((D, CPAD), dtype=np.float32)
        wt[:, :CSH] = sh.T * FP8_PRESCALE
        in_maps.append(
            {"wnT": _to_dr_layout(wt, CPAD), "featnT": featnT8}
        )

    nc = _build()
    res = run_bass_kernel_spmd(nc, in_maps, list(range(NCORES)))
    _last_results = res

    # ---- host combine ----
    S = np.zeros(B, dtype=np.float64)
    for i in range(NCORES):
        sc = np.asarray(res.results[i]["out"], dtype=np.float64)  # [128, 4]
        for bc in range(NBC):
            S[bc * 128:(bc + 1) * 128] += sc[:, bc]

    Z = S + corr
    per = SHIFT + np.log(Z) - t_logit
    loss = float((coef * per).sum())
    return np.array(loss, dtype=np.float32)



# revision 42
# speedup vs baseline: 1.2609x; 1.0484x over previous
"""AdaFace loss on 8 TRN2 NeuronCores, class-parallel.

Strategy: shard the 100k weight rows (classes) across 8 cores. Host
pre-normalizes rows, transposes to [D, C_shard], scales by 8 and casts to
fp8e4 (scale keeps values out of the e4m3 subnormal range; the ScalarE
exp absorbs it: exp(0.5*x - 32) of the 64*cos matmul result). Since
|logit| <= 32, a fixed shift of 32 replaces the per-row max of a
standard log-softmax, so no max collective is needed. Each core returns
per-batch-chunk partial sums of exp(32c-32); the host does the final
O(B) combine: sum across cores, margin-target correction (cos(theta+m)
needs only sqrt, no arccos), ln, weighted dot. No device collective.

Device per core: the whole fp8 weight shard stays resident in SBUF
(24.5KB/partition); DoubleRow matmuls (K=256 per instruction) fill
4-bank PSUM tiles [128b, 2048c]; ScalarE exp -> bf16; VectorE 2x-mode
running adds + one final reduce per batch chunk; single DMA out.
"""

import numpy as np
import ml_dtypes

import concourse.bass as bass
import concourse.tile as tile
from concourse import bacc, mybir
from concourse.bass_utils import run_bass_kernel_spmd

B = 512
D = 256
C = 100000
NCORES = 8
CSH = C // NCORES          # 12500 classes per core
# class tiles per core: six 2048-wide + one 256-wide = 12544.
# Most batch chunks put the 256-wide tile first (cheap pipeline starter);
# the last chunk puts it last so the wide columns can reduce early.
TILES_SMALL_FIRST = [(12288, 256)] + [(i * 2048, 2048) for i in range(6)]
TILES_SMALL_LAST = [(i * 2048, 2048) for i in range(6)] + [(12288, 256)]
CPAD = 12544
NPAD_TOT = (CPAD - CSH) * NCORES

M0 = 0.5
M_MIN = 0.25
SCALE = 32.0
SHIFT = 32.0               # fixed log-softmax shift (|logits| <= SCALE)
FP8_PRESCALE = 8.0         # both operands scaled by 8 -> matmul gives 64*cos

# Schraudolph fake-exp: from x = 64*cos, bf16 bits of exp(0.5*x - 32)
# are i16 = rint(x*FA + FB_EFF); HW rounds to nearest on f32->i16
LOG2E = 1.4426950408889634
FA = 64.0 * LOG2E
FB = 16256.0 - 4096.0 * LOG2E

f32 = mybir.dt.float32
bf16 = mybir.dt.bfloat16
i16 = mybir.dt.int16
fp8 = mybir.dt.float8e4

NBC = B // 128             # 4 batch chunks

_cached_nc = None
_last_results = None


def _schraudolph_rho(fb):
    t = np.linspace(-60.0, -1.0, 200001)
    x = (t + 32.0) * 2.0
    y = np.float32(x) * np.float32(FA) + np.float32(fb)
    i = np.rint(y).astype(np.int16)
    v = i.view(ml_dtypes.bfloat16).astype(np.float64)
    return float(np.mean(v / np.exp(t)))


# fold the mean fake/real ratio into the offset (de-bias)
FB_EFF = FB - 128.0 * np.log2(_schraudolph_rho(FB))
FB_EFF = FB_EFF - 128.0 * np.log2(_schraudolph_rho(FB_EFF))


def _build():
    global _cached_nc
    if _cached_nc is not None:
        return _cached_nc

    nc = bacc.Bacc(
        "TRN2", target_bir_lowering=False, debug=False, num_devices=NCORES
    )

    # [p, j, c] with contraction index k = j*128 + p
    wnT_d = nc.dram_tensor("wnT", [128, 2, CPAD], fp8, kind="ExternalInput")
    featnT_d = nc.dram_tensor("featnT", [128, 2, B], fp8, kind="ExternalInput")
    out_d = nc.dram_tensor("out", [128, NBC], f32, kind="ExternalOutput")

    with tile.TileContext(nc) as tc:
        with (
            tc.tile_pool(name="persist", bufs=1) as persist,
            tc.tile_pool(name="epool", bufs=6) as epool,
            tc.tile_pool(name="psum", bufs=2, space="PSUM") as psum,
        ):
            fsb = persist.tile([128, 2, B], fp8)
            nc.sync.dma_start(out=fsb[:], in_=featnT_d[:])

            wsb = persist.tile([128, 2, CPAD], fp8)
            # chunked loads in consumption order: the small tile's columns
            # first (gates the pipeline start), then the wide region.
            # Spread across the three DMA-capable engines' queues — a single
            # queue serializes at ~50GB/s and trickles for the whole kernel.
            # (engine, lo, hi) in consumption order across the two HWDGE
            # queues (gpsimd's SWDGE path is too slow for bulk loads)
            plan = [
                (nc.scalar, 12288, 12544),
                (nc.scalar, 0, 1536),
                (nc.sync, 1536, 3072),
                (nc.scalar, 3072, 4608),
                (nc.sync, 4608, 6144),
                (nc.scalar, 6144, 7680),
                (nc.sync, 7680, 9216),
                (nc.scalar, 9216, 10752),
                (nc.sync, 10752, 12288),
            ]
            for eng, lo, hi in plan:
                eng.dma_start(
                    out=wsb[:, :, lo:hi], in_=wnT_d[:, :, lo:hi]
                )

            bias_s = persist.tile([128, 1], f32)
            nc.gpsimd.memset(bias_s[:], -SHIFT)

            eacc = [
                persist.tile(
                    [128, 2048], bf16, tag=f"eacc{bc}", name=f"eacc{bc}"
                )
                for bc in range(NBC)
            ]

            S_all = persist.tile([128, NBC], f32)

            S_main = persist.tile([128, 1], f32)

            for bc in range(NBC):
                last = bc == NBC - 1
                tiles = TILES_SMALL_LAST if last else TILES_SMALL_FIRST
                lhs = fsb[:, :, bc * 128:(bc + 1) * 128]
                for ti, (c0, cw) in enumerate(tiles):
                    ps = psum.tile([128, 2048], f32, tag="ps")
                    for j in range(0, cw, 512):
                        jw = min(512, cw - j)
                        nc.tensor.matmul(
                            ps[:, j:j + jw],
                            lhs,
                            wsb[:, :, c0 + j:c0 + j + jw],
                            start=True, stop=True,
                            perf_mode=mybir.MatmulPerfMode.DoubleRow,
                        )
                    if c0 == 10240 and not last:
                        # fake-exp drain on the vector engine: affine to
                        # i16 whose bits are bf16(exp), then add as usual
                        fi = epool.tile([128, 2048], i16, tag="fi")
                        nc.vector.tensor_scalar(
                            fi[:], ps[:],
                            FA, FB_EFF,
                            mybir.AluOpType.mult, mybir.AluOpType.add,
                        )
                        nc.vector.tensor_add(
                            eacc[bc][:], eacc[bc][:], fi[:].bitcast(bf16)
                        )
                        continue
                    esc = epool.tile([128, 2048], bf16, tag="esc")
                    nc.scalar.activation(
                        esc[:, :cw], ps[:, :cw],
                        mybir.ActivationFunctionType.Exp,
                        bias=bias_s[:], scale=SCALE / (FP8_PRESCALE**2),
                    )
                    if ti == 0:
                        nc.vector.tensor_copy(
                            eacc[bc][:, :cw], esc[:, :cw]
                        )
                    elif ti == 1 and not last:
                        nc.vector.tensor_add(
                            eacc[bc][:, :256], eacc[bc][:, :256], esc[:, :256]
                        )
                        nc.vector.tensor_copy(
                            eacc[bc][:, 256:], esc[:, 256:]
                        )
                    else:
                        nc.vector.tensor_add(
                            eacc[bc][:, :cw], eacc[bc][:, :cw], esc[:, :cw]
                        )
                    if bc > 0 and ti == 1:
                        nc.vector.tensor_reduce(
                            S_all[:, bc - 1:bc],
                            eacc[bc - 1][:],
                            axis=mybir.AxisListType.X,
                            op=mybir.AluOpType.add,
                        )
                    if last and ti == len(tiles) - 2:
                        nc.vector.tensor_reduce(
                            S_main[:],
                            eacc[bc][:, 256:2048],
                            axis=mybir.AxisListType.X,
                            op=mybir.AluOpType.add,
                        )

            S_small = persist.tile([128, 1], f32)
            nc.vector.tensor_reduce(
                S_small[:],
                eacc[NBC - 1][:, 0:256],
                axis=mybir.AxisListType.X,
                op=mybir.AluOpType.add,
            )
            nc.vector.tensor_add(S_all[:, NBC - 1:NBC], S_main[:], S_small[:])

            nc.sync.dma_start(out=out_d[:], in_=S_all[:])

    nc.compile()
    _cached_nc = nc
    return nc


def _host_prep(features, weight, weights, labels):
    """Everything O(B*D) / O(C*D) that is not the big matmul."""
    f = features.astype(np.float64)
    norms = np.sqrt((f * f).sum(axis=1))
    lo, hi = norms.min(), norms.max()
    denom = max(hi - lo, 1e-8)
    margins = np.clip(M_MIN + (M0 - M_MIN) * (norms - lo) / denom, M_MIN, M0)
    feat_n = f / np.maximum(norms, 1e-12)[:, None]

    wlab = weight[labels].astype(np.float64)
    wlab_n = wlab / np.maximum(
        np.sqrt((wlab * wlab).sum(axis=1)), 1e-12
    )[:, None]
    cos_t = np.clip((feat_n * wlab_n).sum(axis=1), -1.0 + 1e-7, 1.0 - 1e-7)
    cos_m = cos_t * np.cos(margins) - np.sqrt(1.0 - cos_t * cos_t) * np.sin(
        margins
    )
    t_logit = SCALE * cos_m
    corr = (
        np.exp(SCALE * cos_m - SHIFT)
        - np.exp(SCALE * cos_t - SHIFT)
        - NPAD_TOT * np.exp(-SHIFT)
    )
    coef = weights.astype(np.float64) / B
    return feat_n, corr, coef, t_logit


def _to_dr_layout(mat_t, width):
    """[D, X] f32 -> [128, 2, X] fp8 with k = j*128 + p."""
    a = mat_t.reshape(2, 128, width)          # [j, p, X]
    a = np.ascontiguousarray(a.transpose(1, 0, 2))  # [p, j, X]
    return a.astype(ml_dtypes.float8_e4m3)


def kernel(features, weight, weights, labels):
    global _last_results
    features = np.asarray(features, dtype=np.float32)
    weight = np.asarray(weight, dtype=np.float32)
    weights = np.asarray(weights, dtype=np.float32)
    labels = np.asarray(labels).astype(np.int64)

    feat_n, corr, coef, t_logit = _host_prep(features, weight, weights, labels)

    wn = weight / np.maximum(
        np.linalg.norm(weight, axis=1, keepdims=True), 1e-12
    )
    featnT = np.ascontiguousarray(feat_n.T.astype(np.float32)) * FP8_PRESCALE
    featnT8 = _to_dr_layout(featnT, B)

    in_maps = []
    for i in range(NCORES):
        sh = wn[i * CSH:(i + 1) * CSH]  # [CSH, D]
        wt = np.zeros((D, CPAD), dtype=np.float32)
        wt[:, :CSH] = sh.T * FP8_PRESCALE
        in_maps.append(
            {"wnT": _to_dr_layout(wt, CPAD), "featnT": featnT8}
        )

    nc = _build()
    res = run_bass_kernel_spmd(nc, in_maps, list(range(NCORES)))
    _last_results = res

    # ---- host combine ----
    S = np.zeros(B, dtype=np.float64)
    for i in range(NCORES):
        sc = np.asarray(res.results[i]["out"], dtype=np.float64)  # [128, 4]
        for bc in range(NBC):
            S[bc * 128:(bc + 1) * 128] += sc[:, bc]

    Z = S + corr
    per = SHIFT + np.log(Z) - t_logit
    loss = float((coef * per).sum())
    return np.array(loss, dtype=np.float32)

